# revision 5
# baseline (speedup 1.0000x reference)
"""AevaLinearAttention Trainium2 kernel — 8-way tensor-parallel over heads.

Layout strategy (per core c):
  - owns heads (2c, 2c+1)  -> hidden column block [256c, 256c+256) of Wq/Wk/Wv
  - host passes x pre-transposed (xT [2048, 8192]) so it serves directly as
    the matmul moving operand (d-major projections) without on-device
    transposition of x.
  - q/k/v computed d-major ([d, seq]); feature map elu+1 = exp(min(.,0)) + max(.,0)
  - chunked linear-attention scan per (batch, head): per 128-chunk
      A^T   = matmul(lhsT=kfT, rhs=qfT)            (d contraction)
      inter = matmul(lhsT=qfT_chunk, rhs=S_aug)    (S kept in a PSUM bank,
      intra = matmul(lhsT=masked A^T, rhs=v_aug)    accumulated across chunks)
    with an augmented ones-column on v / normalizer column on S so the
    normalizers z_inter/z_intra ride along as column 128.
  - att chunks transposed on PE and written to a DRAM AllToAll buffer;
    an 8-way AllToAll redistributes head-blocks -> row-blocks, then each core
    computes its 1024-row slice of the output projection with the full Wo.
All matmuls run in float32r (FP22 truncated fp32, full PE rate).
"""

import numpy as np

from concourse import bacc, tile, mybir
from concourse.bass_utils import run_bass_kernel_spmd

B, S, H, D = 2, 4096, 16, 128
HID = H * D                 # 2048
C = 128                     # chunk length
EPS = 1e-6

NCORES = 8
H_LOC = H // NCORES         # 2 heads per core
DBLK = H_LOC * D            # 256 hidden cols per core
SEQ = B * S                 # 8192 global rows
RPC = SEQ // NCORES         # 1024 rows per core
NCH = S // C                # 32 chunks per batch
KT = HID // 128             # 16 k-tiles of contraction

F32 = mybir.dt.float32
F32R = mybir.dt.float32r
ALU = mybir.AluOpType
ACTF = mybir.ActivationFunctionType

_CACHE = {}


def _build_program():
    nc = bacc.Bacc(
        "TRN2", target_bir_lowering=False, debug=False, num_devices=NCORES
    )

    xt_d = nc.dram_tensor("xt", [HID, SEQ], F32R, kind="ExternalInput").ap()
    wq_d = nc.dram_tensor("wq", [HID, DBLK], F32R, kind="ExternalInput").ap()
    wk_d = nc.dram_tensor("wk", [HID, DBLK], F32R, kind="ExternalInput").ap()
    wv_d = nc.dram_tensor("wv", [HID, DBLK], F32R, kind="ExternalInput").ap()
    bq_d = nc.dram_tensor("bq", [128, H_LOC], F32, kind="ExternalInput").ap()
    bk_d = nc.dram_tensor("bk", [128, H_LOC], F32, kind="ExternalInput").ap()
    bv_d = nc.dram_tensor("bv", [128, H_LOC], F32, kind="ExternalInput").ap()
    wo_d = nc.dram_tensor("wo", [HID, HID], F32R, kind="ExternalInput").ap()
    bo_d = nc.dram_tensor("bo", [1, HID], F32R, kind="ExternalInput").ap()
    id_d = nc.dram_tensor("ident", [128, 128], F32R, kind="ExternalInput").ap()
    on1_d = nc.dram_tensor("ones1", [1, 128], F32R, kind="ExternalInput").ap()
    onc_d = nc.dram_tensor("onescol", [128, 2], F32R, kind="ExternalInput").ap()
    mk_d = nc.dram_tensor("maskt", [128, 128], F32R, kind="ExternalInput").ap()
    out_d = nc.dram_tensor("out_rows", [RPC, HID], F32, kind="ExternalOutput").ap()

    with tile.TileContext(nc) as tc:
        with tc.tile_pool(name="misc", bufs=1) as miscp, \
             tc.tile_pool(name="dram", bufs=1, space="DRAM") as dramp:
            bo_sb = miscp.tile([1, HID], F32R)
            nc.sync.dma_start(out=bo_sb[:], in_=bo_d[:])
            ones1 = miscp.tile([1, 128], F32R)
            nc.sync.dma_start(out=ones1[:], in_=on1_d[:])
            onescol = miscp.tile([128, 2], F32R)
            nc.sync.dma_start(out=onescol[:], in_=onc_d[:])

            a2a_in = dramp.tile([NCORES, DBLK, RPC], F32R)
            a2a_out = dramp.tile([NCORES, DBLK, RPC], F32R)

            with tc.tile_pool(name="projw", bufs=1) as projp:
                wq_sb = projp.tile([128, KT, DBLK], F32R)
                wk_sb = projp.tile([128, KT, DBLK], F32R)
                wv_sb = projp.tile([128, KT, DBLK], F32R)
                for t in range(KT):
                    nc.sync.dma_start(out=wq_sb[:, t, :], in_=wq_d[t * 128:(t + 1) * 128, :])
                    nc.sync.dma_start(out=wk_sb[:, t, :], in_=wk_d[t * 128:(t + 1) * 128, :])
                    nc.sync.dma_start(out=wv_sb[:, t, :], in_=wv_d[t * 128:(t + 1) * 128, :])
                bq_sb = projp.tile([128, H_LOC], F32)
                bk_sb = projp.tile([128, H_LOC], F32)
                bv_sb = projp.tile([128, H_LOC], F32)
                nc.sync.dma_start(out=bq_sb[:], in_=bq_d[:])
                nc.sync.dma_start(out=bk_sb[:], in_=bk_d[:])
                nc.sync.dma_start(out=bv_sb[:], in_=bv_d[:])
                ident = projp.tile([128, 128], F32R)
                maskt = projp.tile([128, 128], F32R)
                nc.sync.dma_start(out=ident[:], in_=id_d[:])
                nc.sync.dma_start(out=maskt[:], in_=mk_d[:])

                for b in range(B):
                    with tc.tile_pool(name=f"bt{b}", bufs=1) as bp:
                        # d-major per-head feature-mapped q/k and v (+bias)
                        qf_sb = bp.tile([128, H_LOC, S], F32R)
                        kf_sb = bp.tile([128, H_LOC, S], F32R)
                        vT_sb = bp.tile([128, H_LOC, S], F32R)

                        # ---- projections ----
                        with tc.tile_pool(name=f"pp{b}", bufs=1, space="PSUM") as pps, \
                             tc.tile_pool(name=f"px{b}", bufs=4) as pxp, \
                             tc.tile_pool(name=f"pt{b}", bufs=2) as ptp:
                            for si in range(S // 512):
                                ps_q = [pps.tile([128, 512], F32, tag=f"psq{h}", name=f"ps_q{h}") for h in range(H_LOC)]
                                ps_k = [pps.tile([128, 512], F32, tag=f"psk{h}", name=f"ps_k{h}") for h in range(H_LOC)]
                                ps_v = [pps.tile([128, 512], F32, tag=f"psv{h}", name=f"ps_v{h}") for h in range(H_LOC)]
                                for k in range(KT):
                                    xt_t = pxp.tile([128, 512], F32R, tag="xt")
                                    nc.sync.dma_start(
                                        out=xt_t[:],
                                        in_=xt_d[k * 128:(k + 1) * 128,
                                                 b * S + si * 512: b * S + (si + 1) * 512],
                                    )
                                    st, sp = (k == 0), (k == KT - 1)
                                    for h in range(H_LOC):
                                        hs = slice(h * 128, (h + 1) * 128)
                                        nc.tensor.matmul(ps_q[h][:], wq_sb[:, k, hs], xt_t[:], start=st, stop=sp)
                                        nc.tensor.matmul(ps_k[h][:], wk_sb[:, k, hs], xt_t[:], start=st, stop=sp)
                                        nc.tensor.matmul(ps_v[h][:], wv_sb[:, k, hs], xt_t[:], start=st, stop=sp)
                                ss = slice(si * 512, (si + 1) * 512)
                                for h in range(H_LOC):
                                    # feature map: f(x) = exp(min(x+b,0)) + max(x+b,0)
                                    for ps, bias, dst in (
                                        (ps_q[h], bq_sb, qf_sb),
                                        (ps_k[h], bk_sb, kf_sb),
                                    ):
                                        tmin = ptp.tile([128, 512], F32R, tag="tmin")
                                        nc.vector.tensor_scalar(
                                            tmin[:], ps[:], bias[:, h:h + 1], 0.0,
                                            ALU.add, ALU.min)
                                        texp = ptp.tile([128, 512], F32R, tag="texp")
                                        nc.scalar.activation(texp[:], tmin[:], ACTF.Exp)
                                        tmax = ptp.tile([128, 512], F32R, tag="tmax")
                                        nc.vector.tensor_scalar(
                                            tmax[:], ps[:], bias[:, h:h + 1], 0.0,
                                            ALU.add, ALU.max)
                                        nc.vector.tensor_tensor(
                                            dst[:, h, ss], texp[:], tmax[:], ALU.add)
                                    nc.vector.tensor_scalar(
                                        vT_sb[:, h, ss], ps_v[h][:], bv_sb[:, h:h + 1], None,
                                        ALU.add)

                        # ---- chunked scan ----
                        with tc.tile_pool(name=f"sc{b}", bufs=2) as scp, \
                             tc.tile_pool(name=f"scp{b}", bufs=1, space="PSUM") as sps:
                            for h in range(H_LOC):
                                S_ps = sps.tile([128, 132], F32, tag=f"sps{h}")
                                for n in range(NCH):
                                    cs = slice(n * 128, (n + 1) * 128)
                                    qfT_c = qf_sb[:, h, cs]
                                    kfT_c = kf_sb[:, h, cs]
                                    vT_c = vT_sb[:, h, cs]
                                    # kf chunk -> seq-major
                                    ps_kf = sps.tile([128, 128], F32R, tag="pskf")
                                    nc.tensor.transpose(ps_kf[:], kfT_c, ident[:])
                                    kf_seq = scp.tile([128, 128], F32R, tag="kfseq")
                                    nc.vector.tensor_copy(kf_seq[:], ps_kf[:])
                                    # v chunk -> seq-major with ones column
                                    ps_v2 = sps.tile([128, 128], F32R, tag="psv2")
                                    nc.tensor.transpose(ps_v2[:], vT_c, ident[:])
                                    v_aug = scp.tile([128, 132], F32R, tag="vaug")
                                    nc.vector.tensor_copy(v_aug[:, 0:128], ps_v2[:])
                                    nc.vector.tensor_copy(v_aug[:, 128:130], onescol[:])
                                    # A^T = kf @ qf^T  [k, c]
                                    ps_A = sps.tile([128, 128], F32, tag="psA")
                                    nc.tensor.matmul(ps_A[:], kfT_c, qfT_c, start=True, stop=True)
                                    Am = scp.tile([128, 128], F32R, tag="Am")
                                    nc.vector.tensor_tensor(Am[:], ps_A[:], maskt[:], ALU.mult)
                                    # out_aug = qf @ S_aug + Am^T @ v_aug
                                    ps_O = sps.tile([128, 132], F32, tag="psO")
                                    if n == 0:
                                        nc.tensor.matmul(ps_O[:, 0:130], Am[:], v_aug[:, 0:130],
                                                         start=True, stop=True)
                                    else:
                                        S_sb = scp.tile([128, 132], F32R, tag="ssb")
                                        nc.vector.tensor_copy(S_sb[:, 0:130], S_ps[:, 0:130])
                                        nc.tensor.matmul(ps_O[:, 0:130], qfT_c, S_sb[:, 0:130],
                                                         start=True, stop=False)
                                        nc.tensor.matmul(ps_O[:, 0:130], Am[:], v_aug[:, 0:130],
                                                         start=False, stop=True)
                                    # state += kf^T @ v_aug   (after S_sb copy)
                                    nc.tensor.matmul(S_ps[:, 0:130], kf_seq[:], v_aug[:, 0:130],
                                                     start=(n == 0), stop=(n == NCH - 1),
                                                     skip_group_check=True)
                                    # normalize: att = out / (z + eps)
                                    denom = scp.tile([128, 1], F32, tag="den")
                                    nc.vector.tensor_scalar_add(denom[:], ps_O[:, 128:129], EPS)
                                    rden = scp.tile([128, 1], F32, tag="rden")
                                    nc.vector.reciprocal(rden[:], denom[:])
                                    att = scp.tile([128, 128], F32R, tag="att")
                                    nc.vector.tensor_scalar(att[:], ps_O[:, 0:128],
                                                            rden[:, 0:1], None, ALU.mult)
                                    # transpose att and ship to a2a buffer
                                    ps_at = sps.tile([128, 128], F32R, tag="psat")
                                    nc.tensor.transpose(ps_at[:], att[:], ident[:])
                                    attT = scp.tile([128, 128], F32R, tag="attT")
                                    nc.vector.tensor_copy(attT[:], ps_at[:])
                                    g0 = b * S + n * 128
                                    j, col0 = divmod(g0, RPC)
                                    nc.sync.dma_start(
                                        out=a2a_in[j, h * 128:(h + 1) * 128, col0:col0 + 128],
                                        in_=attT[:])

            # ---- AllToAll: head blocks -> row blocks ----
            nc.gpsimd.collective_compute(
                "AllToAll", ALU.bypass,
                replica_groups=[list(range(NCORES))],
                ins=[a2a_in[:].opt()],
                outs=[a2a_out[:].opt()],
            )

            # ---- output projection for this core's 1024 rows ----
            with tc.tile_pool(name="pd", bufs=1) as pdp, \
                 tc.tile_pool(name="pdo", bufs=2) as pdop, \
                 tc.tile_pool(name="pdp", bufs=1, space="PSUM") as pdps:
                attT_all = pdp.tile([128, KT, RPC], F32R)
                for t in range(KT):
                    nc.sync.dma_start(
                        out=attT_all[:, t, :],
                        in_=a2a_out[t // 2, (t % 2) * 128:(t % 2 + 1) * 128, :])
                for half in range(2):
                    wo_h = pdp.tile([128, KT, 1024], F32R, tag="wo")
                    for t in range(KT):
                        nc.sync.dma_start(
                            out=wo_h[:, t, :],
                            in_=wo_d[t * 128:(t + 1) * 128,
                                     half * 1024:(half + 1) * 1024])
                    for s in range(RPC // 128):
                        ps_o = [pdps.tile([128, 512], F32, tag=f"pso{i}", name=f"ps_o{i}") for i in range(2)]
                        for k in range(KT):
                            for i in range(2):
                                nc.tensor.matmul(
                                    ps_o[i][:],
                                    attT_all[:, k, s * 128:(s + 1) * 128],
                                    wo_h[:, k, i * 512:(i + 1) * 512],
                                    start=(k == 0), stop=False)
                        for i in range(2):
                            nc.tensor.matmul(
                                ps_o[i][:], ones1[:],
                                bo_sb[0:1, half * 1024 + i * 512: half * 1024 + (i + 1) * 512],
                                start=False, stop=True)
                            o_sb = pdop.tile([128, 512], F32, tag="osb")
                            nc.vector.tensor_copy(o_sb[:], ps_o[i][:])
                            nc.sync.dma_start(
                                out=out_d[s * 128:(s + 1) * 128,
                                          half * 1024 + i * 512: half * 1024 + (i + 1) * 512],
                                in_=o_sb[:])

    nc.compile()
    return nc


def _get_program():
    if "nc" not in _CACHE:
        _CACHE["nc"] = _build_program()
    return _CACHE["nc"]


def kernel(x, Wq, bq, Wk, bk, Wv, bv, Wo, bo, _trace=False):
    nc = _get_program()

    x = np.asarray(x, np.float32)
    xt = np.ascontiguousarray(x.reshape(SEQ, HID).T)
    ident = np.eye(128, dtype=np.float32)
    # A^T[k, c] keeps k <= c  -> upper triangular (incl. diagonal)
    maskt = np.triu(np.ones((128, 128), np.float32))

    def shard_bias(bv_):
        return [np.ascontiguousarray(
            np.asarray(bv_, np.float32)[c * DBLK:(c + 1) * DBLK].reshape(H_LOC, 128).T)
            for c in range(NCORES)]

    bqs, bks, bvs = shard_bias(bq), shard_bias(bk), shard_bias(bv)
    wo_full = np.ascontiguousarray(np.asarray(Wo, np.float32))
    bo_row = np.ascontiguousarray(np.asarray(bo, np.float32).reshape(1, HID))

    in_maps = []
    for c in range(NCORES):
        cols = slice(c * DBLK, (c + 1) * DBLK)
        in_maps.append({
            "xt": xt,
            "wq": np.ascontiguousarray(np.asarray(Wq, np.float32)[:, cols]),
            "wk": np.ascontiguousarray(np.asarray(Wk, np.float32)[:, cols]),
            "wv": np.ascontiguousarray(np.asarray(Wv, np.float32)[:, cols]),
            "bq": bqs[c], "bk": bks[c], "bv": bvs[c],
            "wo": wo_full, "bo": bo_row,
            "ident": ident, "maskt": maskt,
            "ones1": np.ones((1, 128), np.float32),
            "onescol": np.ones((128, 2), np.float32),
        })

    res = run_bass_kernel_spmd(nc, in_maps, list(range(NCORES)), trace=_trace)
    out = np.concatenate([res.results[c]["out_rows"] for c in range(NCORES)], axis=0)
    ret = out.reshape(B, S, HID)
    if _trace:
        return ret, res
    return ret


# revision 6
# speedup vs baseline: 1.1227x; 1.1227x over previous
"""AevaLinearAttention Trainium2 kernel — 8-way tensor-parallel over heads.

Layout strategy (per core c):
  - owns heads (2c, 2c+1)  -> hidden column block [256c, 256c+256) of Wq/Wk/Wv
  - host passes x pre-transposed (xT [2048, 8192]) so it serves directly as
    the matmul moving operand (d-major projections) without on-device
    transposition of x.
  - q/k/v computed d-major ([d, seq]); feature map elu+1 = exp(min(.,0)) + max(.,0)
  - chunked linear-attention scan per (batch, head): per 128-chunk
      A^T   = matmul(lhsT=kfT, rhs=qfT)            (d contraction)
      inter = matmul(lhsT=qfT_chunk, rhs=S_aug)    (S kept in a PSUM bank,
      intra = matmul(lhsT=masked A^T, rhs=v_aug)    accumulated across chunks)
    with an augmented ones-column on v / normalizer column on S so the
    normalizers z_inter/z_intra ride along as column 128.
  - att chunks transposed on PE and written to a DRAM AllToAll buffer;
    an 8-way AllToAll redistributes head-blocks -> row-blocks, then each core
    computes its 1024-row slice of the output projection with the full Wo.
Matmul operands are bf16 (FWL weight loads, full PE rate at N=128);
all accumulation is fp32 in PSUM, normalization epilogue in fp32.
"""

import numpy as np
import ml_dtypes

from concourse import bacc, tile, mybir
from concourse.bass_utils import run_bass_kernel_spmd

B, S, H, D = 2, 4096, 16, 128
HID = H * D                 # 2048
C = 128                     # chunk length
EPS = 1e-6

NCORES = 8
H_LOC = H // NCORES         # 2 heads per core
DBLK = H_LOC * D            # 256 hidden cols per core
SEQ = B * S                 # 8192 global rows
RPC = SEQ // NCORES         # 1024 rows per core
NCH = S // C                # 32 chunks per batch
KT = HID // 128             # 16 k-tiles of contraction

F32 = mybir.dt.float32
BF16 = mybir.dt.bfloat16
ALU = mybir.AluOpType
ACTF = mybir.ActivationFunctionType
NPBF16 = ml_dtypes.bfloat16

_CACHE = {}


def _build_program():
    nc = bacc.Bacc(
        "TRN2", target_bir_lowering=False, debug=False, num_devices=NCORES
    )

    xt_d = nc.dram_tensor("xt", [HID, SEQ], BF16, kind="ExternalInput").ap()
    wq_d = nc.dram_tensor("wq", [HID, DBLK], BF16, kind="ExternalInput").ap()
    wk_d = nc.dram_tensor("wk", [HID, DBLK], BF16, kind="ExternalInput").ap()
    wv_d = nc.dram_tensor("wv", [HID, DBLK], BF16, kind="ExternalInput").ap()
    bq_d = nc.dram_tensor("bq", [128, H_LOC], F32, kind="ExternalInput").ap()
    bk_d = nc.dram_tensor("bk", [128, H_LOC], F32, kind="ExternalInput").ap()
    bv_d = nc.dram_tensor("bv", [128, H_LOC], F32, kind="ExternalInput").ap()
    wo_d = nc.dram_tensor("wo", [HID, HID], BF16, kind="ExternalInput").ap()
    bo_d = nc.dram_tensor("bo", [1, HID], BF16, kind="ExternalInput").ap()
    id_d = nc.dram_tensor("ident", [128, 128], BF16, kind="ExternalInput").ap()
    mk_d = nc.dram_tensor("maskt", [128, 128], BF16, kind="ExternalInput").ap()
    on1_d = nc.dram_tensor("ones1", [1, 128], BF16, kind="ExternalInput").ap()
    onc_d = nc.dram_tensor("onescol", [128, 2], BF16, kind="ExternalInput").ap()
    out_d = nc.dram_tensor("out_rows", [RPC, HID], F32, kind="ExternalOutput").ap()

    with tile.TileContext(nc) as tc:
        with tc.tile_pool(name="misc", bufs=1) as miscp, \
             tc.tile_pool(name="dram", bufs=1, space="DRAM") as dramp:
            bo_sb = miscp.tile([1, HID], BF16)
            nc.sync.dma_start(out=bo_sb[:], in_=bo_d[:])
            ones1 = miscp.tile([1, 128], BF16)
            nc.sync.dma_start(out=ones1[:], in_=on1_d[:])
            onescol = miscp.tile([128, 2], BF16)
            nc.sync.dma_start(out=onescol[:], in_=onc_d[:])

            a2a_in = dramp.tile([NCORES, DBLK, RPC], BF16)
            a2a_out = dramp.tile([NCORES, DBLK, RPC], BF16)

            with tc.tile_pool(name="projw", bufs=1) as projp:
                wq_sb = projp.tile([128, KT, DBLK], BF16)
                wk_sb = projp.tile([128, KT, DBLK], BF16)
                wv_sb = projp.tile([128, KT, DBLK], BF16)
                for t in range(KT):
                    nc.sync.dma_start(out=wq_sb[:, t, :], in_=wq_d[t * 128:(t + 1) * 128, :])
                    nc.sync.dma_start(out=wk_sb[:, t, :], in_=wk_d[t * 128:(t + 1) * 128, :])
                    nc.sync.dma_start(out=wv_sb[:, t, :], in_=wv_d[t * 128:(t + 1) * 128, :])
                bq_sb = projp.tile([128, H_LOC], F32)
                bk_sb = projp.tile([128, H_LOC], F32)
                bv_sb = projp.tile([128, H_LOC], F32)
                nc.sync.dma_start(out=bq_sb[:], in_=bq_d[:])
                nc.sync.dma_start(out=bk_sb[:], in_=bk_d[:])
                nc.sync.dma_start(out=bv_sb[:], in_=bv_d[:])
                ident = projp.tile([128, 128], BF16)
                maskt = projp.tile([128, 128], BF16)
                nc.sync.dma_start(out=ident[:], in_=id_d[:])
                nc.sync.dma_start(out=maskt[:], in_=mk_d[:])

                for b in range(B):
                    with tc.tile_pool(name=f"bt{b}", bufs=1) as bp:
                        # d-major per-head feature-mapped q/k and v (+bias)
                        qf_sb = bp.tile([128, H_LOC, S], BF16)
                        kf_sb = bp.tile([128, H_LOC, S], BF16)
                        vT_sb = bp.tile([128, H_LOC, S], BF16)

                        # ---- projections ----
                        with tc.tile_pool(name=f"pp{b}", bufs=1, space="PSUM") as pps, \
                             tc.tile_pool(name=f"px{b}", bufs=4) as pxp, \
                             tc.tile_pool(name=f"pt{b}", bufs=2) as ptp:
                            for si in range(S // 512):
                                ps_q = [pps.tile([128, 512], F32, tag=f"psq{h}", name=f"ps_q{h}") for h in range(H_LOC)]
                                ps_k = [pps.tile([128, 512], F32, tag=f"psk{h}", name=f"ps_k{h}") for h in range(H_LOC)]
                                ps_v = [pps.tile([128, 512], F32, tag=f"psv{h}", name=f"ps_v{h}") for h in range(H_LOC)]
                                for k in range(KT):
                                    xt_t = pxp.tile([128, 512], BF16, tag="xt")
                                    nc.sync.dma_start(
                                        out=xt_t[:],
                                        in_=xt_d[k * 128:(k + 1) * 128,
                                                 b * S + si * 512: b * S + (si + 1) * 512],
                                    )
                                    st, sp = (k == 0), (k == KT - 1)
                                    for h in range(H_LOC):
                                        hs = slice(h * 128, (h + 1) * 128)
                                        nc.tensor.matmul(ps_q[h][:], wq_sb[:, k, hs], xt_t[:], start=st, stop=sp)
                                        nc.tensor.matmul(ps_k[h][:], wk_sb[:, k, hs], xt_t[:], start=st, stop=sp)
                                        nc.tensor.matmul(ps_v[h][:], wv_sb[:, k, hs], xt_t[:], start=st, stop=sp)
                                ss = slice(si * 512, (si + 1) * 512)
                                for h in range(H_LOC):
                                    # feature map: f(x) = exp(min(x+b,0)) + max(x+b,0)
                                    for ps, bias, dst in (
                                        (ps_q[h], bq_sb, qf_sb),
                                        (ps_k[h], bk_sb, kf_sb),
                                    ):
                                        tmin = ptp.tile([128, 512], BF16, tag="tmin")
                                        nc.vector.tensor_scalar(
                                            tmin[:], ps[:], bias[:, h:h + 1], 0.0,
                                            ALU.add, ALU.min)
                                        texp = ptp.tile([128, 512], BF16, tag="texp")
                                        nc.scalar.activation(texp[:], tmin[:], ACTF.Exp)
                                        tmax = ptp.tile([128, 512], BF16, tag="tmax")
                                        nc.vector.tensor_scalar(
                                            tmax[:], ps[:], bias[:, h:h + 1], 0.0,
                                            ALU.add, ALU.max)
                                        nc.vector.tensor_tensor(
                                            dst[:, h, ss], texp[:], tmax[:], ALU.add)
                                    nc.vector.tensor_scalar(
                                        vT_sb[:, h, ss], ps_v[h][:], bv_sb[:, h:h + 1], None,
                                        ALU.add)

                        # ---- chunked scan ----
                        with tc.tile_pool(name=f"sc{b}", bufs=2) as scp, \
                             tc.tile_pool(name=f"scp{b}", bufs=1, space="PSUM") as sps:
                            for h in range(H_LOC):
                                S_ps = sps.tile([128, 132], F32, tag=f"sps{h}", name=f"S_ps{h}")
                                for n in range(NCH):
                                    cs = slice(n * 128, (n + 1) * 128)
                                    qfT_c = qf_sb[:, h, cs]
                                    kfT_c = kf_sb[:, h, cs]
                                    vT_c = vT_sb[:, h, cs]
                                    # kf chunk -> seq-major
                                    ps_kf = sps.tile([128, 128], BF16, tag="pskf")
                                    nc.tensor.transpose(ps_kf[:], kfT_c, ident[:])
                                    kf_seq = scp.tile([128, 128], BF16, tag="kfseq")
                                    nc.vector.tensor_copy(kf_seq[:], ps_kf[:])
                                    # v chunk -> seq-major with ones column
                                    ps_v2 = sps.tile([128, 128], BF16, tag="psv2")
                                    nc.tensor.transpose(ps_v2[:], vT_c, ident[:])
                                    v_aug = scp.tile([128, 132], BF16, tag="vaug")
                                    nc.vector.tensor_copy(v_aug[:, 0:128], ps_v2[:])
                                    nc.vector.tensor_copy(v_aug[:, 128:130], onescol[:])
                                    # A^T = kf @ qf^T  [k, c]
                                    ps_A = sps.tile([128, 128], F32, tag="psA")
                                    nc.tensor.matmul(ps_A[:], kfT_c, qfT_c, start=True, stop=True)
                                    Am = scp.tile([128, 128], BF16, tag="Am")
                                    nc.vector.tensor_tensor(Am[:], ps_A[:], maskt[:], ALU.mult)
                                    # out_aug = qf @ S_aug + Am^T @ v_aug
                                    ps_O = sps.tile([128, 132], F32, tag="psO")
                                    if n == 0:
                                        nc.tensor.matmul(ps_O[:, 0:130], Am[:], v_aug[:, 0:130],
                                                         start=True, stop=True)
                                    else:
                                        S_sb = scp.tile([128, 132], BF16, tag="ssb")
                                        nc.vector.tensor_copy(S_sb[:, 0:130], S_ps[:, 0:130])
                                        nc.tensor.matmul(ps_O[:, 0:130], qfT_c, S_sb[:, 0:130],
                                                         start=True, stop=False)
                                        nc.tensor.matmul(ps_O[:, 0:130], Am[:], v_aug[:, 0:130],
                                                         start=False, stop=True)
                                    # state += kf^T @ v_aug   (after S_sb copy)
                                    nc.tensor.matmul(S_ps[:, 0:130], kf_seq[:], v_aug[:, 0:130],
                                                     start=(n == 0), stop=(n == NCH - 1),
                                                     skip_group_check=True)
                                    # normalize: att = out / (z + eps)
                                    denom = scp.tile([128, 1], F32, tag="den")
                                    nc.vector.tensor_scalar_add(denom[:], ps_O[:, 128:129], EPS)
                                    rden = scp.tile([128, 1], F32, tag="rden")
                                    nc.vector.reciprocal(rden[:], denom[:])
                                    att = scp.tile([128, 128], BF16, tag="att")
                                    nc.vector.tensor_scalar(att[:], ps_O[:, 0:128],
                                                            rden[:, 0:1], None, ALU.mult)
                                    # transpose att and ship to a2a buffer
                                    ps_at = sps.tile([128, 128], BF16, tag="psat")
                                    nc.tensor.transpose(ps_at[:], att[:], ident[:])
                                    attT = scp.tile([128, 128], BF16, tag="attT")
                                    nc.vector.tensor_copy(attT[:], ps_at[:])
                                    g0 = b * S + n * 128
                                    j, col0 = divmod(g0, RPC)
                                    nc.sync.dma_start(
                                        out=a2a_in[j, h * 128:(h + 1) * 128, col0:col0 + 128],
                                        in_=attT[:])

            # ---- AllToAll: head blocks -> row blocks ----
            nc.gpsimd.collective_compute(
                "AllToAll", ALU.bypass,
                replica_groups=[list(range(NCORES))],
                ins=[a2a_in[:].opt()],
                outs=[a2a_out[:].opt()],
            )

            # ---- output projection for this core's 1024 rows ----
            with tc.tile_pool(name="pd", bufs=1) as pdp, \
                 tc.tile_pool(name="pdo", bufs=2) as pdop, \
                 tc.tile_pool(name="pdp", bufs=1, space="PSUM") as pdps:
                attT_all = pdp.tile([128, KT, RPC], BF16)
                for t in range(KT):
                    nc.sync.dma_start(
                        out=attT_all[:, t, :],
                        in_=a2a_out[t // 2, (t % 2) * 128:(t % 2 + 1) * 128, :])
                for half in range(2):
                    wo_h = pdp.tile([128, KT, 1024], BF16, tag="wo")
                    for t in range(KT):
                        nc.sync.dma_start(
                            out=wo_h[:, t, :],
                            in_=wo_d[t * 128:(t + 1) * 128,
                                     half * 1024:(half + 1) * 1024])
                    for s in range(RPC // 128):
                        ps_o = [pdps.tile([128, 512], F32, tag=f"pso{i}", name=f"ps_o{i}") for i in range(2)]
                        for k in range(KT):
                            for i in range(2):
                                nc.tensor.matmul(
                                    ps_o[i][:],
                                    attT_all[:, k, s * 128:(s + 1) * 128],
                                    wo_h[:, k, i * 512:(i + 1) * 512],
                                    start=(k == 0), stop=False)
                        for i in range(2):
                            nc.tensor.matmul(
                                ps_o[i][:], ones1[:],
                                bo_sb[0:1, half * 1024 + i * 512: half * 1024 + (i + 1) * 512],
                                start=False, stop=True)
                            o_sb = pdop.tile([128, 512], F32, tag="osb")
                            nc.vector.tensor_copy(o_sb[:], ps_o[i][:])
                            nc.sync.dma_start(
                                out=out_d[s * 128:(s + 1) * 128,
                                          half * 1024 + i * 512: half * 1024 + (i + 1) * 512],
                                in_=o_sb[:])

    nc.compile()
    return nc


def _get_program():
    if "nc" not in _CACHE:
        _CACHE["nc"] = _build_program()
    return _CACHE["nc"]


def kernel(x, Wq, bq, Wk, bk, Wv, bv, Wo, bo, _trace=False):
    nc = _get_program()

    x = np.asarray(x, np.float32)
    xt = np.ascontiguousarray(x.reshape(SEQ, HID).T.astype(NPBF16))
    ident = np.eye(128, dtype=NPBF16)
    # A^T[k, c] keeps k <= c  -> upper triangular (incl. diagonal)
    maskt = np.triu(np.ones((128, 128), NPBF16))

    def shard_bias(bv_):
        return [np.ascontiguousarray(
            np.asarray(bv_, np.float32)[c * DBLK:(c + 1) * DBLK].reshape(H_LOC, 128).T)
            for c in range(NCORES)]

    bqs, bks, bvs = shard_bias(bq), shard_bias(bk), shard_bias(bv)
    wo_full = np.ascontiguousarray(np.asarray(Wo, np.float32).astype(NPBF16))
    bo_row = np.ascontiguousarray(np.asarray(bo, np.float32).reshape(1, HID).astype(NPBF16))

    in_maps = []
    for c in range(NCORES):
        cols = slice(c * DBLK, (c + 1) * DBLK)
        in_maps.append({
            "xt": xt,
            "wq": np.ascontiguousarray(np.asarray(Wq, np.float32)[:, cols].astype(NPBF16)),
            "wk": np.ascontiguousarray(np.asarray(Wk, np.float32)[:, cols].astype(NPBF16)),
            "wv": np.ascontiguousarray(np.asarray(Wv, np.float32)[:, cols].astype(NPBF16)),
            "bq": bqs[c], "bk": bks[c], "bv": bvs[c],
            "wo": wo_full, "bo": bo_row,
            "ident": ident, "maskt": maskt,
            "ones1": np.ones((1, 128), NPBF16),
            "onescol": np.ones((128, 2), NPBF16),
        })

    res = run_bass_kernel_spmd(nc, in_maps, list(range(NCORES)), trace=_trace)
    out = np.concatenate([res.results[c]["out_rows"] for c in range(NCORES)], axis=0)
    ret = out.reshape(B, S, HID)
    if _trace:
        return ret, res
    return ret


# revision 7
# speedup vs baseline: 1.2010x; 1.0697x over previous
"""AevaLinearAttention Trainium2 kernel — 8-way tensor-parallel over heads.

Layout strategy (per core c):
  - owns heads (2c, 2c+1)  -> hidden column block [256c, 256c+256) of Wq/Wk/Wv
  - host passes x pre-transposed (xT [2048, 8192]) so it serves directly as
    the matmul moving operand (d-major projections) without on-device
    transposition of x.
  - q/k/v computed d-major ([d, seq]); feature map elu+1 = exp(min(.,0)) + max(.,0)
  - chunked linear-attention scan per (batch, head): per 128-chunk
      A^T   = matmul(lhsT=kfT, rhs=qfT)            (d contraction)
      inter = matmul(lhsT=qfT_chunk, rhs=S_aug)    (S kept in a PSUM bank,
      intra = matmul(lhsT=masked A^T, rhs=v_aug)    accumulated across chunks)
    with an augmented ones-column on v / normalizer column on S so the
    normalizers z_inter/z_intra ride along as column 128.
  - att chunks transposed on PE and written to a per-batch DRAM AllToAll
    buffer; the batch-0 AllToAll overlaps batch-1 compute. Each collective
    redistributes head-blocks -> 512-row blocks; each core then computes its
    (512 rows of batch 0) + (512 rows of batch 1) slice of the output
    projection with the full Wo (preloaded in SBUF).
Matmul operands are bf16 (FWL weight loads, full PE rate at N=128);
all accumulation is fp32 in PSUM, normalization epilogue in fp32.
"""

import numpy as np
import ml_dtypes

from concourse import bacc, tile, mybir
from concourse.bass_utils import run_bass_kernel_spmd

B, S, H, D = 2, 4096, 16, 128
HID = H * D                 # 2048
C = 128                     # chunk length
EPS = 1e-6

NCORES = 8
H_LOC = H // NCORES         # 2 heads per core
DBLK = H_LOC * D            # 256 hidden cols per core
SEQ = B * S                 # 8192 global rows
RPB = S // NCORES           # 512 rows per core per batch
NCH = S // C                # 32 chunks per batch
KT = HID // 128             # 16 k-tiles of contraction

F32 = mybir.dt.float32
BF16 = mybir.dt.bfloat16
ALU = mybir.AluOpType
ACTF = mybir.ActivationFunctionType
NPBF16 = ml_dtypes.bfloat16

_CACHE = {}


def _build_program():
    nc = bacc.Bacc(
        "TRN2", target_bir_lowering=False, debug=False, num_devices=NCORES
    )

    xt_d = nc.dram_tensor("xt", [HID, SEQ], BF16, kind="ExternalInput").ap()
    wq_d = nc.dram_tensor("wq", [HID, DBLK], BF16, kind="ExternalInput").ap()
    wk_d = nc.dram_tensor("wk", [HID, DBLK], BF16, kind="ExternalInput").ap()
    wv_d = nc.dram_tensor("wv", [HID, DBLK], BF16, kind="ExternalInput").ap()
    bq_d = nc.dram_tensor("bq", [128, H_LOC], F32, kind="ExternalInput").ap()
    bk_d = nc.dram_tensor("bk", [128, H_LOC], F32, kind="ExternalInput").ap()
    bv_d = nc.dram_tensor("bv", [128, H_LOC], F32, kind="ExternalInput").ap()
    wo_d = nc.dram_tensor("wo", [HID, HID], BF16, kind="ExternalInput").ap()
    bo_d = nc.dram_tensor("bo", [1, HID], BF16, kind="ExternalInput").ap()
    id_d = nc.dram_tensor("ident", [128, 128], BF16, kind="ExternalInput").ap()
    mk_d = nc.dram_tensor("maskt", [128, 128], BF16, kind="ExternalInput").ap()
    on1_d = nc.dram_tensor("ones1", [1, 128], BF16, kind="ExternalInput").ap()
    onc_d = nc.dram_tensor("onescol", [128, 2], BF16, kind="ExternalInput").ap()
    # rows 0:512 = this core's 512-row slice of batch 0, rows 512:1024 batch 1
    out_d = nc.dram_tensor("out_rows", [B * RPB, HID], F32, kind="ExternalOutput").ap()

    with tile.TileContext(nc) as tc:
        with tc.tile_pool(name="misc", bufs=1) as miscp, \
             tc.tile_pool(name="dram", bufs=1, space="DRAM") as dramp:
            bo_sb = miscp.tile([1, HID], BF16)
            nc.sync.dma_start(out=bo_sb[:], in_=bo_d[:])
            ones1 = miscp.tile([1, 128], BF16)
            nc.sync.dma_start(out=ones1[:], in_=on1_d[:])
            onescol = miscp.tile([128, 2], BF16)
            nc.sync.dma_start(out=onescol[:], in_=onc_d[:])
            # full Wo resident; DMA overlaps the batch phases
            wo_sb = miscp.tile([128, KT, HID], BF16)
            for t in range(KT):
                nc.scalar.dma_start(out=wo_sb[:, t, :], in_=wo_d[t * 128:(t + 1) * 128, :])
            # received attT slices land here as each batch collective finishes
            attT_all = miscp.tile([128, KT, B * RPB], BF16)

            a2a_in = [dramp.tile([NCORES, DBLK, RPB], BF16, name=f"a2a_in{b}")
                      for b in range(B)]
            a2a_out = [dramp.tile([NCORES, DBLK, RPB], BF16, name=f"a2a_out{b}")
                      for b in range(B)]

            with tc.tile_pool(name="projw", bufs=1) as projp:
                bq_sb = projp.tile([128, H_LOC], F32)
                bk_sb = projp.tile([128, H_LOC], F32)
                bv_sb = projp.tile([128, H_LOC], F32)
                nc.sync.dma_start(out=bq_sb[:], in_=bq_d[:])
                nc.sync.dma_start(out=bk_sb[:], in_=bk_d[:])
                nc.sync.dma_start(out=bv_sb[:], in_=bv_d[:])
                ident = projp.tile([128, 128], BF16)
                maskt = projp.tile([128, 128], BF16)
                nc.sync.dma_start(out=ident[:], in_=id_d[:])
                nc.sync.dma_start(out=maskt[:], in_=mk_d[:])
                wq_sb = projp.tile([128, KT, DBLK], BF16)
                wk_sb = projp.tile([128, KT, DBLK], BF16)
                wv_sb = projp.tile([128, KT, DBLK], BF16)
                nc.sync.dma_start(out=wq_sb[:], in_=wq_d.rearrange("(t p) c -> p t c", p=128))
                nc.sync.dma_start(out=wk_sb[:], in_=wk_d.rearrange("(t p) c -> p t c", p=128))
                nc.sync.dma_start(out=wv_sb[:], in_=wv_d.rearrange("(t p) c -> p t c", p=128))

                for b in range(B):
                    with tc.tile_pool(name=f"bt{b}", bufs=1) as bp:
                        # d-major per-head feature-mapped q/k and v (+bias)
                        qf_sb = bp.tile([128, H_LOC, S], BF16)
                        kf_sb = bp.tile([128, H_LOC, S], BF16)
                        vT_sb = bp.tile([128, H_LOC, S], BF16)

                        # ---- projections ----
                        with tc.tile_pool(name=f"pp{b}", bufs=1, space="PSUM") as pps, \
                             tc.tile_pool(name=f"px{b}", bufs=4) as pxp, \
                             tc.tile_pool(name=f"pt{b}", bufs=2) as ptp:
                            for si in range(S // 512):
                                ps_q = [pps.tile([128, 512], F32, tag=f"psq{h}", name=f"ps_q{h}") for h in range(H_LOC)]
                                ps_k = [pps.tile([128, 512], F32, tag=f"psk{h}", name=f"ps_k{h}") for h in range(H_LOC)]
                                ps_v = [pps.tile([128, 512], F32, tag=f"psv{h}", name=f"ps_v{h}") for h in range(H_LOC)]
                                for k in range(KT):
                                    xt_t = pxp.tile([128, 512], BF16, tag="xt")
                                    nc.sync.dma_start(
                                        out=xt_t[:],
                                        in_=xt_d[k * 128:(k + 1) * 128,
                                                 b * S + si * 512: b * S + (si + 1) * 512],
                                    )
                                    st, sp = (k == 0), (k == KT - 1)
                                    for h in range(H_LOC):
                                        hs = slice(h * 128, (h + 1) * 128)
                                        nc.tensor.matmul(ps_q[h][:], wq_sb[:, k, hs], xt_t[:], start=st, stop=sp)
                                        nc.tensor.matmul(ps_k[h][:], wk_sb[:, k, hs], xt_t[:], start=st, stop=sp)
                                        nc.tensor.matmul(ps_v[h][:], wv_sb[:, k, hs], xt_t[:], start=st, stop=sp)
                                ss = slice(si * 512, (si + 1) * 512)
                                for h in range(H_LOC):
                                    # feature map: f(x) = exp(min(x+b,0)) + max(x+b,0)
                                    for ps, bias, dst in (
                                        (ps_q[h], bq_sb, qf_sb),
                                        (ps_k[h], bk_sb, kf_sb),
                                    ):
                                        tmin = ptp.tile([128, 512], BF16, tag="tmin")
                                        nc.vector.tensor_scalar(
                                            tmin[:], ps[:], bias[:, h:h + 1], 0.0,
                                            ALU.add, ALU.min)
                                        texp = ptp.tile([128, 512], BF16, tag="texp")
                                        nc.scalar.activation(texp[:], tmin[:], ACTF.Exp)
                                        tmax = ptp.tile([128, 512], BF16, tag="tmax")
                                        nc.vector.tensor_scalar(
                                            tmax[:], ps[:], bias[:, h:h + 1], 0.0,
                                            ALU.add, ALU.max)
                                        nc.vector.tensor_tensor(
                                            dst[:, h, ss], texp[:], tmax[:], ALU.add)
                                    nc.vector.tensor_scalar(
                                        vT_sb[:, h, ss], ps_v[h][:], bv_sb[:, h:h + 1], None,
                                        ALU.add)

                        # ---- chunked scan ----
                        with tc.tile_pool(name=f"sc{b}", bufs=2) as scp, \
                             tc.tile_pool(name=f"scp{b}", bufs=1, space="PSUM") as sps:
                            for h in range(H_LOC):
                                S_ps = sps.tile([128, 132], F32, tag=f"sps{h}", name=f"S_ps{h}")
                                for n in range(NCH):
                                    cs = slice(n * 128, (n + 1) * 128)
                                    qfT_c = qf_sb[:, h, cs]
                                    kfT_c = kf_sb[:, h, cs]
                                    vT_c = vT_sb[:, h, cs]
                                    # kf chunk -> seq-major (copies on ScalarE:
                                    # DVE is the scan's busiest engine)
                                    ps_kf = sps.tile([128, 128], BF16, tag="pskf")
                                    nc.tensor.transpose(ps_kf[:], kfT_c, ident[:])
                                    kf_seq = scp.tile([128, 128], BF16, tag="kfseq")
                                    nc.scalar.activation(kf_seq[:], ps_kf[:], ACTF.Copy)
                                    # v chunk -> seq-major with ones column
                                    ps_v2 = sps.tile([128, 128], BF16, tag="psv2")
                                    nc.tensor.transpose(ps_v2[:], vT_c, ident[:])
                                    v_aug = scp.tile([128, 132], BF16, tag="vaug")
                                    nc.scalar.activation(v_aug[:, 0:128], ps_v2[:], ACTF.Copy)
                                    nc.vector.tensor_copy(v_aug[:, 128:130], onescol[:])
                                    # A^T = kf @ qf^T  [k, c]
                                    ps_A = sps.tile([128, 128], F32, tag="psA")
                                    nc.tensor.matmul(ps_A[:], kfT_c, qfT_c, start=True, stop=True)
                                    Am = scp.tile([128, 128], BF16, tag="Am")
                                    nc.vector.tensor_tensor(Am[:], ps_A[:], maskt[:], ALU.mult)
                                    # out_aug = qf @ S_aug + Am^T @ v_aug
                                    ps_O = sps.tile([128, 132], F32, tag="psO", bufs=2)
                                    if n == 0:
                                        nc.tensor.matmul(ps_O[:, 0:130], Am[:], v_aug[:, 0:130],
                                                         start=True, stop=True)
                                    else:
                                        S_sb = scp.tile([128, 132], BF16, tag="ssb")
                                        nc.vector.tensor_copy(S_sb[:, 0:130], S_ps[:, 0:130])
                                        nc.tensor.matmul(ps_O[:, 0:130], qfT_c, S_sb[:, 0:130],
                                                         start=True, stop=False)
                                        nc.tensor.matmul(ps_O[:, 0:130], Am[:], v_aug[:, 0:130],
                                                         start=False, stop=True)
                                    # state += kf^T @ v_aug   (after S_sb copy)
                                    nc.tensor.matmul(S_ps[:, 0:130], kf_seq[:], v_aug[:, 0:130],
                                                     start=(n == 0), stop=(n == NCH - 1),
                                                     skip_group_check=True)
                                    # normalize: att = out / (z + eps)
                                    denom = scp.tile([128, 1], F32, tag="den")
                                    nc.vector.tensor_scalar_add(denom[:], ps_O[:, 128:129], EPS)
                                    rden = scp.tile([128, 1], F32, tag="rden")
                                    nc.vector.reciprocal(rden[:], denom[:])
                                    att = scp.tile([128, 128], BF16, tag="att")
                                    nc.vector.tensor_scalar(att[:], ps_O[:, 0:128],
                                                            rden[:, 0:1], None, ALU.mult)
                                    # transpose att and ship to a2a buffer
                                    ps_at = sps.tile([128, 128], BF16, tag="psat")
                                    nc.tensor.transpose(ps_at[:], att[:], ident[:])
                                    attT = scp.tile([128, 128], BF16, tag="attT")
                                    nc.vector.tensor_copy(attT[:], ps_at[:])
                                    j, col0 = divmod(n * 128, RPB)
                                    nc.sync.dma_start(
                                        out=a2a_in[b][j, h * 128:(h + 1) * 128, col0:col0 + 128],
                                        in_=attT[:])

                    # AllToAll for this batch: head blocks -> 512-row blocks.
                    # The batch-0 collective overlaps batch-1 compute.
                    nc.gpsimd.collective_compute(
                        "AllToAll", ALU.bypass,
                        replica_groups=[list(range(NCORES))],
                        ins=[a2a_in[b][:].opt()],
                        outs=[a2a_out[b][:].opt()],
                    )
                    for t in range(KT):
                        nc.scalar.dma_start(
                            out=attT_all[:, t, b * RPB:(b + 1) * RPB],
                            in_=a2a_out[b][t // 2, (t % 2) * 128:(t % 2 + 1) * 128, :])

            # ---- output projection: (512 batch-0 + 512 batch-1) rows ----
            with tc.tile_pool(name="pdo", bufs=2) as pdop, \
                 tc.tile_pool(name="pdp", bufs=1, space="PSUM") as pdps:
                for half in range(2):
                    for s in range(B * RPB // 128):
                        ps_o = [pdps.tile([128, 512], F32, tag=f"pso{i}", name=f"ps_o{i}") for i in range(2)]
                        for k in range(KT):
                            for i in range(2):
                                nc.tensor.matmul(
                                    ps_o[i][:],
                                    attT_all[:, k, s * 128:(s + 1) * 128],
                                    wo_sb[:, k, half * 1024 + i * 512: half * 1024 + (i + 1) * 512],
                                    start=(k == 0), stop=False)
                        for i in range(2):
                            nc.tensor.matmul(
                                ps_o[i][:], ones1[:],
                                bo_sb[0:1, half * 1024 + i * 512: half * 1024 + (i + 1) * 512],
                                start=False, stop=True)
                            o_sb = pdop.tile([128, 512], F32, tag="osb")
                            nc.vector.tensor_copy(o_sb[:], ps_o[i][:])
                            nc.sync.dma_start(
                                out=out_d[s * 128:(s + 1) * 128,
                                          half * 1024 + i * 512: half * 1024 + (i + 1) * 512],
                                in_=o_sb[:])

    nc.compile()
    return nc


def _get_program():
    if "nc" not in _CACHE:
        _CACHE["nc"] = _build_program()
    return _CACHE["nc"]


def kernel(x, Wq, bq, Wk, bk, Wv, bv, Wo, bo, _trace=False):
    nc = _get_program()

    x = np.asarray(x, np.float32)
    xt = np.ascontiguousarray(x.reshape(SEQ, HID).T.astype(NPBF16))
    ident = np.eye(128, dtype=NPBF16)
    # A^T[k, c] keeps k <= c  -> upper triangular (incl. diagonal)
    maskt = np.triu(np.ones((128, 128), NPBF16))

    def shard_bias(bv_):
        return [np.ascontiguousarray(
            np.asarray(bv_, np.float32)[c * DBLK:(c + 1) * DBLK].reshape(H_LOC, 128).T)
            for c in range(NCORES)]

    bqs, bks, bvs = shard_bias(bq), shard_bias(bk), shard_bias(bv)
    wo_full = np.ascontiguousarray(np.asarray(Wo, np.float32).astype(NPBF16))
    bo_row = np.ascontiguousarray(np.asarray(bo, np.float32).reshape(1, HID).astype(NPBF16))

    in_maps = []
    for c in range(NCORES):
        cols = slice(c * DBLK, (c + 1) * DBLK)
        in_maps.append({
            "xt": xt,
            "wq": np.ascontiguousarray(np.asarray(Wq, np.float32)[:, cols].astype(NPBF16)),
            "wk": np.ascontiguousarray(np.asarray(Wk, np.float32)[:, cols].astype(NPBF16)),
            "wv": np.ascontiguousarray(np.asarray(Wv, np.float32)[:, cols].astype(NPBF16)),
            "bq": bqs[c], "bk": bks[c], "bv": bvs[c],
            "wo": wo_full, "bo": bo_row,
            "ident": ident, "maskt": maskt,
            "ones1": np.ones((1, 128), NPBF16),
            "onescol": np.ones((128, 2), NPBF16),
        })

    res = run_bass_kernel_spmd(nc, in_maps, list(range(NCORES)), trace=_trace)
    # core c rows: batch 0 rows [c*512:(c+1)*512] then batch 1 same range
    out = np.empty((B, S, HID), np.float32)
    for c in range(NCORES):
        r = res.results[c]["out_rows"]
        for b in range(B):
            out[b, c * RPB:(c + 1) * RPB] = r[b * RPB:(b + 1) * RPB]
    if _trace:
        return out, res
    return out
